# revision 7
# baseline (speedup 1.0000x reference)
"""Trainium2 Bass kernel: causal multi-head attention (B=4,S=2048,D=1024,H=16).

Sharding (8 cores, no collectives): core c -> batch b=c//2, q-half h=c%2.
Each core computes all 16 heads for 8 interleaved query tiles of 128 rows
(abs q-tile t = 2*j + h for local slot j), plus full K/V for its batch,
and the full fc_out for its own query rows.  The host scatters the 8
per-core [1024,1024] outputs back into [4,2048,1024].

Device pipeline per core (all matmuls bf16, f32 accumulation):
  P1: Q/K/V projections (stationary x^T blocks, moving per-head weights),
      PSUM->SBUF cast + bias, DMA-xbar transposes to build Q^T/K^T.
  P2: per (head, k-tile): scores^T = K^T.T @ Q^T -> PSUM, exp via ScalarE
      (scale=1/8 folded in), 0/1 mask multiply on "mixed" tiles only,
      out^T accumulation with ones-augmented V (row 64 = softmax denom).
      Normalization by the reciprocal of the denominator at head end.
  P3: fc_out = concat^T.T @ Wo + bo for the local query rows.

The program is specialized at build time to the mask's block structure
(skip all-zero blocks / skip masking on all-ones blocks); this is computed
from the actual mask input, so it stays correct for any mask.
"""

import os
import numpy as np
import ml_dtypes

import concourse.bass as bass
import concourse.mybir as mybir
import concourse.tile as tile
from concourse import bacc
from concourse.bass_utils import run_bass_kernel_spmd

B, S, D, H, HD = 4, 2048, 1024, 16, 64
N_CORES = 8
ST = 128               # tile edge (partition size)
NKT = S // ST          # 16 key tiles
NJ = 8                 # local query slots per core (8*128 = 1024 rows)
NDC = D // ST          # 8 contraction chunks
NG = H // 2            # 8 head pairs (2 heads packed per 128 partitions)

F32 = mybir.dt.float32
BF16 = mybir.dt.bfloat16


def _classify(mask: np.ndarray):
    """Block structure of the mask, unioned over the two q-halves.

    Returns (cls[NJ][NKT] in {0 skip,1 full,2 mixed}, mixed list [(j,k)]).
    """
    cls = np.zeros((NJ, NKT), dtype=int)
    for j in range(NJ):
        for k in range(NKT):
            blocks = [
                mask[(2 * j + h) * ST:(2 * j + h + 1) * ST, k * ST:(k + 1) * ST]
                for h in (0, 1)
            ]
            if all((b != 0).all() for b in blocks):
                cls[j, k] = 1
            elif all((b == 0).all() for b in blocks):
                cls[j, k] = 0
            else:
                cls[j, k] = 2
        # close interior holes so every slot's computed k-range is contiguous
        nz = np.nonzero(cls[j])[0]
        if len(nz):
            for k in range(nz[0], nz[-1] + 1):
                if cls[j, k] == 0:
                    cls[j, k] = 2
    mixed = [(j, k) for j in range(NJ) for k in range(NKT) if cls[j, k] == 2]
    return cls, mixed


def _build(cls, mixed, n_maskt):
    """Build the (uniform, SPMD) Bass program for one core's shard."""
    nc = bacc.Bacc("TRN2", target_bir_lowering=False, debug=False,
                   num_devices=N_CORES)

    x_d = nc.dram_tensor("x", [S, D], F32, kind="ExternalInput")
    xq_d = nc.dram_tensor("xq", [NJ * ST, D], F32, kind="ExternalInput")
    wq_d = nc.dram_tensor("wq", [H, D, HD], F32, kind="ExternalInput")
    wk_d = nc.dram_tensor("wk", [H, D, HD], F32, kind="ExternalInput")
    wv_d = nc.dram_tensor("wv", [H, D, HD], F32, kind="ExternalInput")
    wo_d = nc.dram_tensor("wo", [D, D], F32, kind="ExternalInput")
    bq_d = nc.dram_tensor("bq", [H, HD], F32, kind="ExternalInput")
    bk_d = nc.dram_tensor("bk", [H, HD], F32, kind="ExternalInput")
    bv_d = nc.dram_tensor("bv", [H, HD], F32, kind="ExternalInput")
    bo_d = nc.dram_tensor("bo", [D], F32, kind="ExternalInput")
    mt_d = nc.dram_tensor("maskt", [n_maskt, ST, ST], BF16, kind="ExternalInput")
    out_d = nc.dram_tensor("out", [NJ * ST, D], F32, kind="ExternalOutput")

    mixed_idx = {jk: i for i, jk in enumerate(mixed)}
    # per-k slot spans and per-slot k ranges
    slots_k = [[j for j in range(NJ) if cls[j, k]] for k in range(NKT)]
    kfirst = {}
    klast = {}
    for j in range(NJ):
        ks = [k for k in range(NKT) if cls[j, k]]
        if ks:
            kfirst[j], klast[j] = ks[0], ks[-1]

    NB = NJ // 4  # PSUM 512-col banks per po tile (2)
    bank_slots = [[j for j in range(4 * b_, 4 * b_ + 4) if j in kfirst]
                  for b_ in range(NB)]
    bklast = {b_: max(klast[j] for j in bank_slots[b_])
              for b_ in range(NB) if bank_slots[b_]}
    bank_fast = {b_: len({kfirst[j] for j in bank_slots[b_]}) == 1
                 for b_ in range(NB) if bank_slots[b_]}

    with tile.TileContext(nc) as tc:
        with (
            tc.tile_pool(name="persist", bufs=1) as pp,      # lives whole kernel
        ):
            # ---- persistent SBUF tensors -------------------------------
            kt_t = [pp.tile([ST, S], BF16, name=f"ktg{g}", tag=f"ktg{g}")
                    for g in range(NG)]
            qt_t = [pp.tile([ST, NJ * ST], BF16, name=f"qtg{g}", tag=f"qtg{g}")
                    for g in range(NG)]
            vb = pp.tile([ST, NKT, H, HD + 1], BF16, name="vb", tag="vb")
            cat = [pp.tile([ST, NJ * ST], BF16, name=f"catg{g}", tag=f"catg{g}")
                   for g in range(NG)]
            mtb = pp.tile([ST, max(n_maskt, 1), ST], BF16, name="mtb", tag="mtb")
            bob = pp.tile([ST, D], F32, name="bob", tag="bob")

            nc.vector.memset(vb[:, :, :, HD:HD + 1], 1.0)
            nc.sync.dma_start(mtb[:, :, :], mt_d.ap().rearrange("m p f -> p m f"))
            bo_ap = bo_d.ap()
            nc.sync.dma_start(
                bob[:, :],
                bass.AP(tensor=bo_ap.tensor, offset=bo_ap.offset,
                        ap=[[0, ST]] + list(bo_ap.ap)))

            def load_bias_bcast(pool, bias_d, name):
                t = pool.tile([ST, H, HD], F32, name=name, tag=name, bufs=1)
                src = bias_d.ap()
                nc.sync.dma_start(
                    t[:, :, :],
                    bass.AP(tensor=src.tensor, offset=src.offset,
                            ap=[[0, ST]] + list(src.ap)))
                return t

            def load_w(pool, w_d, tag):
                # layout [p, c, h, e] so one matmul spans 8 heads (512 cols)
                t = pool.tile([ST, NDC, H, HD], BF16, name=tag, tag=tag, bufs=1)
                for h in range(H):
                    src = w_d.ap()[h].rearrange("(c p) e -> p c e", p=ST)
                    wstg = pool.tile([ST, NDC, HD], F32, tag="wstg")
                    nc.sync.dma_start(wstg[:, :, :], src)
                    nc.vector.tensor_copy(t[:, :, h, :], wstg[:, :, :])
                return t

            def load_xt(pool, src_d, row, pfx):
                xf = pool.tile([ST, D], F32, tag="xf")
                nc.sync.dma_start(xf[:, :], src_d.ap()[row * ST:(row + 1) * ST, :])
                xb = pool.tile([ST, D], BF16, tag="xb")
                nc.vector.tensor_copy(xb[:, :], xf[:, :])
                xt = {}
                for c in range(NDC):
                    t = pool.tile([ST, ST], BF16, name=f"{pfx}{c}", tag=f"{pfx}{c}")
                    nc.scalar.dma_start_transpose(
                        t[:, :], xb[:, c * ST:(c + 1) * ST])
                    xt[c] = t
                return xt

            # ---- phase 1a: K/V projections over full sequence ----------
            with (
                tc.tile_pool(name="p1a", bufs=3) as p1a,
                tc.tile_pool(name="ppsa", bufs=4, space="PSUM") as ppsa,
            ):
                wkb = load_w(p1a, wk_d, "wkb")
                wvb = load_w(p1a, wv_d, "wvb")
                bkb = load_bias_bcast(p1a, bk_d, "bkb")
                bvb = load_bias_bcast(p1a, bv_d, "bvb")
                for st in range(NKT):
                    xt = load_xt(p1a, x_d, st, "xt")
                    psk = ppsa.tile([ST, H * HD], F32, tag="ps")
                    psv = ppsa.tile([ST, H * HD], F32, tag="ps")
                    for c in range(NDC):
                        for n in range(2):
                            for ps, wt in ((psk, wkb), (psv, wvb)):
                                nc.tensor.matmul(
                                    ps[:, n * 512:(n + 1) * 512],
                                    xt[c][:, :],
                                    wt[:, c, 8 * n:8 * n + 8, :],
                                    start=(c == 0), stop=(c == NDC - 1))
                    kst = p1a.tile([ST, H * HD], BF16, tag="kst")
                    nc.vector.tensor_add(
                        kst[:, :], psk[:, :],
                        bkb[:, :, :].rearrange("p h e -> p (h e)"))
                    for g in range(NG):
                        nc.scalar.dma_start_transpose(
                            kt_t[g][:, st * ST:(st + 1) * ST],
                            kst[:, g * ST:(g + 1) * ST])
                    nc.vector.tensor_add(
                        vb[:, st, :, 0:HD],
                        psv[:, :].rearrange("p (h e) -> p h e", h=H),
                        bvb[:, :, :])

            # ---- phase 1b: Q projection over local query rows ----------
            with (
                tc.tile_pool(name="p1b", bufs=3) as p1b,
                tc.tile_pool(name="ppsb", bufs=3, space="PSUM") as ppsb,
            ):
                wqb = load_w(p1b, wq_d, "wqb")
                bqb = load_bias_bcast(p1b, bq_d, "bqb")
                for jl in range(NJ):
                    xqt = load_xt(p1b, xq_d, jl, "xqt")
                    psq = ppsb.tile([ST, H * HD], F32, tag="ps")
                    for c in range(NDC):
                        for n in range(2):
                            nc.tensor.matmul(
                                psq[:, n * 512:(n + 1) * 512],
                                xqt[c][:, :],
                                wqb[:, c, 8 * n:8 * n + 8, :],
                                start=(c == 0), stop=(c == NDC - 1))
                    qst = p1b.tile([ST, H * HD], BF16, tag="qst")
                    nc.vector.tensor_add(
                        qst[:, :], psq[:, :],
                        bqb[:, :, :].rearrange("p h e -> p (h e)"))
                    for g in range(NG):
                        nc.scalar.dma_start_transpose(
                            qt_t[g][:, jl * ST:(jl + 1) * ST],
                            qst[:, g * ST:(g + 1) * ST])

            # ---- phase 2: attention ------------------------------------
            with (
                tc.tile_pool(name="p2s", bufs=3) as p2s,
                tc.tile_pool(name="pss", bufs=2, space="PSUM") as pss,
                tc.tile_pool(name="pso", bufs=2, space="PSUM") as pso,
            ):
                for h in range(H):
                    g, r = h // 2, (h % 2) * HD
                    po = pso.tile([HD + 1, NJ * ST], F32, tag="po")
                    for b_ in range(NB):
                        if bank_slots[b_] and not bank_fast[b_]:
                            nc.vector.memset(
                                po[:, b_ * 512:(b_ + 1) * 512], 0.0)
                    for k in range(NKT):
                        sl = slots_k[k]
                        if not sl:
                            continue
                        jlo, jhi = sl[0], sl[-1] + 1
                        psc = pss.tile([ST, NJ * ST], F32, tag="psc")
                        # score chunks: contiguous slot runs split at banks
                        runs = []
                        run = [sl[0]]
                        for j in sl[1:]:
                            if j == run[-1] + 1 and j // 4 == run[0] // 4:
                                run.append(j)
                            else:
                                runs.append(run)
                                run = [j]
                        runs.append(run)
                        for run in runs:
                            ja, jb = run[0], run[-1]
                            nc.tensor.matmul(
                                psc[:, ja * ST:(jb + 1) * ST],
                                kt_t[g][r:r + HD, k * ST:(k + 1) * ST],
                                qt_t[g][r:r + HD, ja * ST:(jb + 1) * ST],
                                start=True, stop=True)
                        pt = p2s.tile([ST, NJ * ST], BF16, tag="pt")
                        nc.scalar.activation(
                            pt[:, jlo * ST:jhi * ST], psc[:, jlo * ST:jhi * ST],
                            mybir.ActivationFunctionType.Exp,
                            scale=1.0 / float(np.sqrt(HD)))
                        for j in sl:
                            if cls[j, k] == 2:
                                m = mixed_idx[(j, k)]
                                nc.vector.tensor_mul(
                                    pt[:, j * ST:(j + 1) * ST],
                                    pt[:, j * ST:(j + 1) * ST],
                                    mtb[:, m, :])
                        # AV chunks: runs also split where kfirst differs
                        for run in runs:
                            sub = [run[0]]
                            subs = []
                            for j in run[1:]:
                                if kfirst[j] == kfirst[sub[0]]:
                                    sub.append(j)
                                else:
                                    subs.append(sub)
                                    sub = [j]
                            subs.append(sub)
                            for sub_ in subs:
                                ja, jb = sub_[0], sub_[-1]
                                b_ = ja // 4
                                fast = bank_fast[b_]
                                nc.tensor.matmul(
                                    po[0:HD + 1, ja * ST:(jb + 1) * ST],
                                    vb[:, k, h, :],
                                    pt[:, ja * ST:(jb + 1) * ST],
                                    start=(fast and k == kfirst[ja]),
                                    stop=(fast and k == bklast[b_]),
                                    skip_group_check=not fast)
                    rec = p2s.tile([1, NJ * ST], F32, tag="rec")
                    nc.vector.reciprocal(rec[:, :], po[HD:HD + 1, :])
                    recb = p2s.tile([HD, NJ * ST], F32, tag="recb")
                    nc.gpsimd.partition_broadcast(recb[:, :], rec[:, :])
                    nc.vector.tensor_mul(cat[g][r:r + HD, :], po[0:HD, :],
                                         recb[:, :])

            # ---- phase 3: fc_out ---------------------------------------
            with (
                tc.tile_pool(name="p3s", bufs=3) as p3s,
                tc.tile_pool(name="psy", bufs=4, space="PSUM") as psy,
            ):
                wob = p3s.tile([ST, NDC, D], BF16, name="wob", tag="wob", bufs=1)
                for c in range(NDC):
                    wstg = p3s.tile([ST, D], F32, tag="wstg3")
                    nc.sync.dma_start(wstg[:, :],
                                      wo_d.ap()[c * ST:(c + 1) * ST, :])
                    nc.vector.tensor_copy(wob[:, c, :], wstg[:, :])
                for jt in range(NJ):
                    for n in range(2):
                        py = psy.tile([ST, 512], F32, tag="py")
                        for c in range(NDC):
                            nc.tensor.matmul(
                                py[:, :],
                                cat[c][:, jt * ST:(jt + 1) * ST],
                                wob[:, c, n * 512:(n + 1) * 512],
                                start=(c == 0), stop=(c == NDC - 1))
                        ysb = p3s.tile([ST, 512], F32, tag="ysb")
                        nc.vector.tensor_add(ysb[:, :], py[:, :],
                                             bob[:, n * 512:(n + 1) * 512])
                        nc.sync.dma_start(
                            out_d.ap()[jt * ST:(jt + 1) * ST,
                                       n * 512:(n + 1) * 512],
                            ysb[:, :])

    nc.compile()
    return nc


_CACHE = {}
LAST_RESULT = None


def _get_program(mask):
    key = mask.tobytes()
    if key not in _CACHE:
        cls, mixed = _classify(mask)
        _CACHE[key] = (_build(cls, mixed, max(len(mixed), 1)), cls, mixed)
    return _CACHE[key]


def kernel(x, mask, Wq, bq, Wk, bk, Wv, bv, Wo, bo):
    x = np.ascontiguousarray(np.asarray(x, dtype=np.float32))
    mask = np.asarray(mask)
    nc, cls, mixed = _get_program(mask)

    n_maskt = max(len(mixed), 1)
    base = {
        "wq": np.ascontiguousarray(Wq, dtype=np.float32),
        "wk": np.ascontiguousarray(Wk, dtype=np.float32),
        "wv": np.ascontiguousarray(Wv, dtype=np.float32),
        "wo": np.ascontiguousarray(Wo, dtype=np.float32),
        "bq": np.ascontiguousarray(bq, dtype=np.float32),
        "bk": np.ascontiguousarray(bk, dtype=np.float32),
        "bv": np.ascontiguousarray(bv, dtype=np.float32),
        "bo": np.ascontiguousarray(bo, dtype=np.float32),
    }
    in_maps = []
    for c in range(N_CORES):
        b, h = c // 2, c % 2
        qrows = np.concatenate(
            [np.arange((2 * j + h) * ST, (2 * j + h + 1) * ST) for j in range(NJ)])
        mt = np.zeros((n_maskt, ST, ST), dtype=ml_dtypes.bfloat16)
        for i, (j, k) in enumerate(mixed):
            blk = mask[(2 * j + h) * ST:(2 * j + h + 1) * ST,
                       k * ST:(k + 1) * ST]
            mt[i] = (blk != 0).T.astype(ml_dtypes.bfloat16)
        m = dict(base)
        m["x"] = x[b]
        m["xq"] = np.ascontiguousarray(x[b][qrows])
        m["maskt"] = mt
        in_maps.append(m)

    res = run_bass_kernel_spmd(
        nc, in_maps, core_ids=list(range(N_CORES)),
        trace=os.environ.get("BASS_KERNEL_TRACE", "0") == "1")
    global LAST_RESULT
    LAST_RESULT = res

    out = np.empty((B, S, D), dtype=np.float32)
    for c in range(N_CORES):
        b, h = c // 2, c % 2
        oc = res.results[c]["out"]
        for j in range(NJ):
            out[b, (2 * j + h) * ST:(2 * j + h + 1) * ST, :] = \
                oc[j * ST:(j + 1) * ST, :]
    return out
